# revision 1
# baseline (speedup 1.0000x reference)
"""Transformer decoder layer (causal self-attn + cross-attn + FFN, 3 post-LNs)
on 8 Trainium2 NeuronCores.

Sharding: 2-way data parallel (batch) x 4-way tensor parallel.
  core c: batch g = c // 4, TP rank r = c % 4.
  - attention: 4 of 16 heads per core (wq/wk/wv column slice 256, wo row
    slice 256), AllReduce[group of 4] after the output projection.
  - FFN: w1 column slice 1024, w2 row slice 1024, AllReduce after w2.
  - residual: each core folds 0.25*residual (+ bias/4) into its partial
    before the AllReduce, so the AllReduce output is directly the LN input.
  - LayerNorms computed redundantly on each core of the group.

On-chip layouts (per core, S tokens):
  feature-major "transposed" activations xT: [128, 8, S] bf16 (E on partitions)
  qT/kT: [128, 2, S] bf16 (head-dim on partitions, 4 heads x 64)
  v:     [128, TB, 4, 65] bf16 token-major, col 64 = ones (rowsum trick)
  attention scores sT: [128 k, 512 q] blocks, softmax along k via exp +
    ones-column rowsums; normalization folded into the o-eviction scale.

Matmul operands are bf16 (f32 PSUM accumulation); residual / LN / collective
payloads are f32.
"""

import numpy as np
import ml_dtypes

import concourse.bass as bass
import concourse.bacc as bacc
import concourse.tile as tile
from concourse import mybir
from concourse import bass_utils
from concourse.masks import make_identity

F32 = mybir.dt.float32
BF16 = mybir.dt.bfloat16
AF = mybir.ActivationFunctionType
ALU = mybir.AluOpType

E = 1024
H_PER_CORE = 4      # heads per core (16 / 4 TP ranks)
DK = 64
QKV = H_PER_CORE * DK   # 256
FFN_SLICE = 1024        # 4096 / 4 TP ranks
EB = E // 128           # 8 E partition-blocks
NEG_BIG = -30000.0      # additive mask value (exp -> 0 in f32)


def _ts(i, n):
    return slice(i * n, (i + 1) * n)


def _pbcast(ap, p=128):
    """Broadcast a 1D DRAM AP across p partitions (partition step 0)."""
    return bass.AP(tensor=ap.tensor, offset=ap.offset, ap=[[0, p]] + list(ap.ap))


def build_decoder_nc(S: int, num_devices: int = 8, stop_after: str | None = None):
    """Build the SPMD Bass program for one core (sequence length S)."""
    assert S % 512 == 0
    TB = S // 128          # token blocks
    QT = S // 512          # query tiles

    nc = bacc.Bacc("TRN2", target_bir_lowering=False, debug=False,
                   num_devices=num_devices)

    # ---------------- DRAM I/O ----------------
    din = {}

    def inp(name, shape, dt):
        din[name] = nc.dram_tensor(name, list(shape), dt, kind="ExternalInput")
        return din[name]

    x0_f = inp("x0_f", [S, E], F32)          # input (this batch), f32
    x0_b = inp("x0_b", [S, E], BF16)         # same, bf16 (for DMA transpose)
    enc_b = inp("enc_b", [S, E], BF16)       # encoder output, bf16

    for p in ("sa", "ca"):
        inp(f"{p}_wq", [E, QKV], BF16)
        inp(f"{p}_wk", [E, QKV], BF16)
        inp(f"{p}_wv", [E, QKV], BF16)
        inp(f"{p}_wo", [QKV, E], BF16)
        inp(f"{p}_bq", [QKV], F32)
        inp(f"{p}_bk", [QKV], F32)
        inp(f"{p}_bv", [QKV], F32)
        inp(f"{p}_bo4", [E], F32)            # bo / group_size
    inp("w1", [E, FFN_SLICE], BF16)
    inp("b1", [FFN_SLICE], F32)
    inp("w2", [FFN_SLICE, E], BF16)
    inp("b24", [E], F32)                     # b2 / group_size
    for i in (1, 2, 3):
        inp(f"ln{i}_g", [E], F32)
        inp(f"ln{i}_b", [E], F32)
    inp("cmask", [4, 128, 512], BF16)        # causal straddle masks

    G = 4 if num_devices >= 8 else num_devices
    out = nc.dram_tensor("out", [S // G, E], F32, kind="ExternalOutput")

    rg = [[0, 1, 2, 3], [4, 5, 6, 7]][: max(1, num_devices // 4)]
    if num_devices < 8:
        rg = [list(range(num_devices))]

    with tile.TileContext(nc) as tc:
        _emit(tc, din, out, S, TB, QT, rg, stop_after)

    nc.compile()
    return nc


PHASES = ["xt", "saqkv", "saattn", "sa", "cakv", "ar1", "ln1",
          "ca", "ar2", "ln2", "ffn1", "ffn2", "full"]


def _emit(tc, din, out, S, TB, QT, rg, stop_after=None):
    nc = tc.nc

    def cut(phase):
        # True -> caller should emit the early-exit and stop
        return stop_after == phase

    with (
        tc.tile_pool(name="const", bufs=1) as const,
        tc.tile_pool(name="wpool", bufs=1) as wpool,
        tc.tile_pool(name="xt", bufs=1) as xt_pool,
        tc.tile_pool(name="qkv", bufs=1) as qkv_pool,
        tc.tile_pool(name="attn", bufs=2) as attn_pool,
        tc.tile_pool(name="opool", bufs=1) as o_pool,
        tc.tile_pool(name="lnp", bufs=2) as lnp,
        tc.tile_pool(name="stat", bufs=8) as stat,
        tc.tile_pool(name="pp", bufs=2, space="PSUM") as pp,
        tc.tile_pool(name="ps_s", bufs=2, space="PSUM") as ps_s,
        tc.tile_pool(name="ps_o", bufs=2, space="PSUM") as ps_o,
        tc.tile_pool(name="ps_t", bufs=2, space="PSUM") as ps_t,
        tc.tile_pool(name="dram", bufs=1, space="DRAM") as dram,
    ):
        # ---------------- constants ----------------
        ident = const.tile([128, 128], BF16)
        make_identity(nc, ident)
        eps_t = const.tile([128, 1], F32)
        nc.vector.memset(eps_t, 1e-12)
        cmask = const.tile([128, 4, 512], BF16)
        nc.sync.dma_start(out=cmask, in_=din["cmask"].ap().rearrange("i p q -> p i q"))

        _bcast_cache = {}

        def bcast(name, dt=F32, tag=""):
            if name in _bcast_cache:
                return _bcast_cache[name]
            t = const.tile([128, E], dt, name=f"bc_{name}", tag=tag)
            nc.sync.dma_start(out=t, in_=_pbcast(din[name].ap()))
            _bcast_cache[name] = t
            return t

        def ln_g(i):
            return bcast(f"ln{i}_g", tag="lng")

        def ln_b(i):
            return bcast(f"ln{i}_b", tag="lnb")

        def bo4(p):
            return bcast(f"{p}_bo4", tag="bo4")

        def b24_b():
            return bcast("b24", tag="bo4")

        # per-partition bias tiles
        def pp_bias(name, nj):
            t = const.tile([128, nj], F32, name=f"ppb_{name}")
            nc.sync.dma_start(out=t, in_=din[name].ap().rearrange("(j p) -> p j", p=128))
            return t

        bq = {p: pp_bias(f"{p}_bq", 2) for p in ("sa", "ca")}
        bk = {p: pp_bias(f"{p}_bk", 2) for p in ("sa", "ca")}
        b1_t = pp_bias("b1", 8)
        def bv_b(p):
            t = const.tile([128, QKV], F32, name=f"bvb_{p}", tag="bvb")
            nc.sync.dma_start(out=t, in_=_pbcast(din[f"{p}_bv"].ap()))
            return t

        # ---------------- DRAM scratch ----------------
        G = len(rg[0])
        ar_in, ar_out = {}, {}
        for i in (1, 2):
            ar_in[i] = dram.tile([S, E], BF16, name=f"ar{i}_in")
            ar_out[i] = dram.tile([S, E], BF16, name=f"ar{i}_out")
        ar_in[3] = dram.tile([S, E], F32, name="ar3_in")
        ar_out[3] = dram.tile([S // G, E], F32, name="rs3_out")
        x_res = {1: dram.tile([S, E], F32, name="x1_dram"),
                 2: dram.tile([S, E], F32, name="x2_dram")}
        x_bf = {1: dram.tile([S, E], BF16, name="x1bf_dram"),
                2: dram.tile([S, E], BF16, name="x2bf_dram")}

        # ---------------- helpers ----------------
        def load_w_qkv(pref):
            w = {}
            for nm in ("wq", "wk", "wv"):
                t = wpool.tile([128, EB, QKV], BF16, tag=nm, name=f"{pref}_{nm}_sb")
                nc.sync.dma_start(out=t, in_=din[f"{pref}_{nm}"].ap().rearrange(
                    "(eb p) m -> p eb m", p=128))
                w[nm] = t
            return w

        def load_w_o(pref):
            t = wpool.tile([128, 2, E], BF16, tag="wo", name=f"{pref}_wo_sb")
            nc.sync.dma_start(out=t, in_=din[f"{pref}_wo"].ap().rearrange(
                "(j p) n -> p j n", p=128))
            return t

        def dma_transpose_in(dst, src_dram):
            # src [S, E] (2-byte) -> dst [128, EB, S] feature-major
            for eb in range(EB):
                nc.sync.dma_start_transpose(dst[:, eb, :], src_dram[:, _ts(eb, 128)])

        def proj_qk(xT, w, b, dst):
            # dst [128, 2, S] bf16 = (w.T @ x.T) + b   (feature-major)
            for j in range(2):
                for tt in range(QT):
                    ps = pp.tile([128, 512], F32, tag="pp")
                    for eb in range(EB):
                        nc.tensor.matmul(ps, w[:, eb, _ts(j, 128)],
                                         xT[:, eb, _ts(tt, 512)],
                                         start=(eb == 0), stop=(eb == EB - 1))
                    nc.vector.tensor_scalar_add(dst[:, j, _ts(tt, 512)], ps,
                                                b[:, j:j + 1])

        def proj_v(xT, w, bvb, dst):
            # dst [128, TB, 4, 65] token-major v (+ ones column)
            nc.vector.memset(dst[:, :, :, 64:65], 1.0)
            for tb in range(TB):
                ps = pp.tile([128, QKV], F32, tag="pp")
                for eb in range(EB):
                    nc.tensor.matmul(ps, xT[:, eb, _ts(tb, 128)], w[:, eb, :],
                                     start=(eb == 0), stop=(eb == EB - 1))
                nc.vector.tensor_add(dst[:, tb, :, 0:64],
                                     ps.rearrange("p (h d) -> p h d", d=64), bvb)

        def attention(qT, kT, v, o_sb, causal):
            for h in range(H_PER_CORE):
                hp = slice((h % 2) * 64, (h % 2) * 64 + 64)
                j = h // 2
                for qt in range(QT):
                    kb_max = min(TB, 4 * qt + 4) if causal else TB
                    at = attn_pool.tile([128, TB, 512], BF16, tag="attn")
                    for kb in range(kb_max):
                        ps = ps_s.tile([128, 512], F32, tag="ps_s")
                        nc.tensor.matmul(ps, kT[hp, j, _ts(kb, 128)],
                                         qT[hp, j, _ts(qt, 512)],
                                         start=True, stop=True)
                        nc.scalar.activation(at[:, kb, :], ps, AF.Exp, scale=0.125)
                        if causal and kb >= 4 * qt:
                            nc.vector.tensor_mul(at[:, kb, :], at[:, kb, :],
                                                 cmask[:, kb - 4 * qt, :])
                    for qs in range(4):
                        po = ps_o.tile([128, 65], F32, tag="ps_o")
                        for kb in range(kb_max):
                            nc.tensor.matmul(po, at[:, kb, _ts(qs, 128)],
                                             v[:, kb, h, :],
                                             start=(kb == 0), stop=(kb == kb_max - 1))
                        rcp = stat.tile([128, 1], F32, tag="rcp")
                        nc.vector.reciprocal(rcp, po[:, 64:65])
                        nc.vector.tensor_scalar_mul(o_sb[:, qt * 4 + qs, h, :],
                                                    po[:, 0:64], rcp)

        def o_transpose(o_sb, oT):
            for tb in range(TB):
                for j in range(2):
                    pt = ps_t.tile([128, 128], BF16, tag="ps_t")
                    nc.tensor.transpose(pt, o_sb[:, tb, 2 * j:2 * j + 2, :], ident)
                    nc.vector.tensor_copy(oT[:, j, _ts(tb, 128)], pt)

        def out_proj(oT, wo, bo4_b, ar_dst):
            # bf16 partial = oT.T @ wo + bo/G -> ar_dst (residual added post-AR)
            for tb in range(TB):
                y = lnp.tile([128, E], BF16, tag="res_out")
                for nh in range(2):
                    ps = pp.tile([128, 512], F32, tag="pp")
                    for j in range(2):
                        nc.tensor.matmul(ps, oT[:, j, _ts(tb, 128)],
                                         wo[:, j, _ts(nh, 512)],
                                         start=(j == 0), stop=(j == 1))
                    nc.vector.tensor_add(y[:, _ts(nh, 512)], ps,
                                         bo4_b[:, _ts(nh, 512)])
                nc.sync.dma_start(out=ar_dst[_ts(tb, 128), :], in_=y)

        def all_reduce(i):
            nc.gpsimd.collective_compute(
                "AllReduce", ALU.add, replica_groups=rg,
                ins=[ar_in[i].opt()], outs=[ar_out[i].opt()])

        def reduce_scatter(i):
            nc.gpsimd.collective_compute(
                "ReduceScatter", ALU.add, replica_groups=rg,
                ins=[ar_in[i].opt()], outs=[ar_out[i].opt()])

        def layer_norm(i, make_bf, to_out=None, residual_src=None, n_blocks=None):
            # LN over (ar_out[i] [+ residual]); writes x_res[i]/x_bf[i] or `out`
            for tb in range(n_blocks if n_blocks is not None else TB):
                ld = lnp.tile([128, E], F32, tag="ln_io")
                if residual_src is not None:
                    arb = lnp.tile([128, E], BF16, tag="ln_bf")
                    nc.sync.dma_start(out=arb, in_=ar_out[i][_ts(tb, 128), :])
                    nc.sync.dma_start(out=ld, in_=residual_src[_ts(tb, 128), :])
                    nc.vector.tensor_add(ld, ld, arb)
                else:
                    nc.sync.dma_start(out=ld, in_=ar_out[i][_ts(tb, 128), :])
                st = stat.tile([128, 2, 6], F32, tag="bnst")
                for sg in range(2):
                    nc.vector.bn_stats(st[:, sg, :], ld[:, _ts(sg, 512)])
                mv = stat.tile([128, 2], F32, tag="bnmv")
                nc.vector.bn_aggr(mv, st)
                sd = stat.tile([128, 1], F32, tag="sd")
                nc.scalar.activation(sd, mv[:, 1:2], AF.Sqrt, bias=eps_t)
                rstd = stat.tile([128, 1], F32, tag="rstd")
                nc.vector.reciprocal(rstd, sd)
                nc.vector.tensor_scalar(ld, ld, mv[:, 0:1], rstd,
                                        ALU.subtract, ALU.mult)
                nc.vector.tensor_mul(ld, ld, ln_g(i))
                nc.vector.tensor_add(ld, ld, ln_b(i))
                xf = ld
                if to_out is not None:
                    nc.sync.dma_start(out=to_out[_ts(tb, 128), :], in_=xf)
                else:
                    nc.sync.dma_start(out=x_res[i][_ts(tb, 128), :], in_=xf)
                    if make_bf:
                        xb = lnp.tile([128, E], BF16, tag="ln_bf")
                        nc.vector.tensor_copy(xb, xf)
                        nc.sync.dma_start(out=x_bf[i][_ts(tb, 128), :], in_=xb)

        # ================= self-attention =================
        def finish():
            nc.sync.dma_start(out=out.ap(), in_=din["x0_f"].ap()[:S // len(rg[0]), :])

        if cut("null"):
            finish()
            return

        x0T = xt_pool.tile([128, EB, S], BF16, tag="xT", name="x0T")
        dma_transpose_in(x0T, din["x0_b"].ap())

        sa_w = load_w_qkv("sa")
        sa_wo = load_w_o("sa")

        qT = qkv_pool.tile([128, 2, S], BF16, tag="qT", name="sa_qT")
        kT = qkv_pool.tile([128, 2, S], BF16, tag="kT", name="sa_kT")
        v = qkv_pool.tile([128, TB, 4, 65], BF16, tag="v", name="sa_v")
        proj_qk(x0T, sa_w["wq"], bq["sa"], qT)
        proj_qk(x0T, sa_w["wk"], bk["sa"], kT)
        proj_v(x0T, sa_w["wv"], bv_b("sa"), v)

        if cut("saqkv"):
            finish()
            return

        # encoder transpose-load takes over x0T's slot once SA projections drain
        encT = xt_pool.tile([128, EB, S], BF16, tag="xT", name="encT")
        dma_transpose_in(encT, din["enc_b"].ap())

        if cut("xt"):
            finish()
            return

        o_sb = o_pool.tile([128, TB, 4, 64], BF16, tag="o", name="sa_o")
        attention(qT, kT, v, o_sb, causal=True)
        oT = qkv_pool.tile([128, 2, S], BF16, tag="qT", name="sa_oT")
        o_transpose(o_sb, oT)

        if cut("saattn"):
            finish()
            return
        out_proj(oT, sa_wo, bo4("sa"), ar_in[1])

        if cut("sa"):
            finish()
            return

        # cross-attention K/V from encoder (independent of AR1 -> overlaps it)
        ca_w = load_w_qkv("ca")
        ca_kT = qkv_pool.tile([128, 2, S], BF16, tag="kT", name="ca_kT")
        ca_v = qkv_pool.tile([128, TB, 4, 65], BF16, tag="v", name="ca_v")
        proj_qk(encT, ca_w["wk"], bk["ca"], ca_kT)
        proj_v(encT, ca_w["wv"], bv_b("ca"), ca_v)

        if cut("cakv"):
            finish()
            return

        all_reduce(1)

        if cut("ar1"):
            finish()
            return
        layer_norm(1, make_bf=True, residual_src=din["x0_f"].ap())

        # ================= cross-attention =================
        x1T = xt_pool.tile([128, EB, S], BF16, tag="xT", name="x1T")
        dma_transpose_in(x1T, x_bf[1])

        if cut("ln1"):
            finish()
            return
        ca_wo = load_w_o("ca")
        ca_qT = qkv_pool.tile([128, 2, S], BF16, tag="qT", name="ca_qT")
        proj_qk(x1T, ca_w["wq"], bq["ca"], ca_qT)

        ca_o = o_pool.tile([128, TB, 4, 64], BF16, tag="o", name="ca_o")
        attention(ca_qT, ca_kT, ca_v, ca_o, causal=False)
        ca_oT = qkv_pool.tile([128, 2, S], BF16, tag="qT", name="ca_oT")
        o_transpose(ca_o, ca_oT)
        out_proj(ca_oT, ca_wo, bo4("ca"), ar_in[2])

        if cut("ca"):
            finish()
            return

        # FFN weights load early (overlaps AR2)
        w1_sb = wpool.tile([128, EB, FFN_SLICE], BF16, tag="wk")
        nc.sync.dma_start(out=w1_sb, in_=din["w1"].ap().rearrange(
            "(eb p) m -> p eb m", p=128))
        w2_sb = wpool.tile([128, 8, E], BF16, tag="wq")
        nc.sync.dma_start(out=w2_sb, in_=din["w2"].ap().rearrange(
            "(fb p) n -> p fb n", p=128))

        all_reduce(2)

        if cut("ar2"):
            finish()
            return
        layer_norm(2, make_bf=True, residual_src=x_res[1])

        # ================= FFN =================
        x2T = xt_pool.tile([128, EB, S], BF16, tag="xT", name="x2T")
        dma_transpose_in(x2T, x_bf[2])

        if cut("ln2"):
            finish()
            return
        hT = xt_pool.tile([128, 8, S], BF16, tag="hT", name="hT")
        for fb in range(8):
            for tt in range(QT):
                ps = pp.tile([128, 512], F32, tag="pp")
                for eb in range(EB):
                    nc.tensor.matmul(ps, w1_sb[:, eb, _ts(fb, 128)],
                                     x2T[:, eb, _ts(tt, 512)],
                                     start=(eb == 0), stop=(eb == EB - 1))
                nc.scalar.activation(hT[:, fb, _ts(tt, 512)], ps, AF.Relu,
                                     bias=b1_t[:, fb:fb + 1])

        if cut("ffn1"):
            finish()
            return
        for tb in range(TB):
            res = lnp.tile([128, E], F32, tag="ln_io")
            nc.sync.dma_start(out=res, in_=x_res[2][_ts(tb, 128), :])
            nc.vector.scalar_tensor_tensor(res, res, 1.0 / len(rg[0]),
                                           b24_b(), ALU.mult, ALU.add)
            base = res
            y = lnp.tile([128, E], F32, tag="res_out")
            for nh in range(2):
                ps = pp.tile([128, 512], F32, tag="pp")
                for fb in range(8):
                    nc.tensor.matmul(ps, hT[:, fb, _ts(tb, 128)],
                                     w2_sb[:, fb, _ts(nh, 512)],
                                     start=(fb == 0), stop=(fb == 7))
                nc.vector.tensor_add(y[:, _ts(nh, 512)], base[:, _ts(nh, 512)], ps)
            nc.sync.dma_start(out=ar_in[3][_ts(tb, 128), :], in_=y)

        if cut("ffn2"):
            finish()
            return

        reduce_scatter(3)
        layer_norm(3, make_bf=False, to_out=out.ap(), n_blocks=TB // G)


# ====================== host side ======================

def make_causal_masks():
    # mask_i[pk, pq] = 1.0 if pk <= pq - 128*i else 0  (straddle blocks)
    m = np.zeros((4, 128, 512), dtype=np.float32)
    pk = np.arange(128)[:, None]
    pq = np.arange(512)[None, :]
    for i in range(4):
        m[i] = (pk <= pq - 128 * i).astype(np.float32)
    return m.astype(ml_dtypes.bfloat16)


def shard_inputs(inputs, num_devices=8):
    """Full inputs (reference.setup_inputs keys) -> per-core in_maps."""
    bf = ml_dtypes.bfloat16
    f32 = np.float32
    G = 4 if num_devices >= 8 else num_devices
    cmask = make_causal_masks()
    in_maps = []
    inp = {k: np.asarray(v) for k, v in inputs.items()}
    for c in range(num_devices):
        g = c // G if num_devices >= 8 else 0
        r = c % G
        qs = slice(r * QKV, (r + 1) * QKV)
        fs = slice(r * FFN_SLICE, (r + 1) * FFN_SLICE)
        x0 = inp["input"][g].astype(f32)
        m = {
            "x0_f": x0,
            "x0_b": x0.astype(bf),
            "enc_b": inp["encoder_output"][g].astype(bf),
            "w1": inp["ffn_w1"][:, fs].astype(bf),
            "b1": inp["ffn_b1"][fs].astype(f32),
            "w2": inp["ffn_w2"][fs, :].astype(bf),
            "b24": (inp["ffn_b2"] / G).astype(f32),
            "cmask": cmask,
        }
        for p in ("sa", "ca"):
            m[f"{p}_wq"] = inp[f"{p}_wq"][:, qs].astype(bf)
            m[f"{p}_wk"] = inp[f"{p}_wk"][:, qs].astype(bf)
            m[f"{p}_wv"] = inp[f"{p}_wv"][:, qs].astype(bf)
            m[f"{p}_wo"] = inp[f"{p}_wo"][qs, :].astype(bf)
            m[f"{p}_bq"] = inp[f"{p}_bq"][qs].astype(f32)
            m[f"{p}_bk"] = inp[f"{p}_bk"][qs].astype(f32)
            m[f"{p}_bv"] = inp[f"{p}_bv"][qs].astype(f32)
            m[f"{p}_bo4"] = (inp[f"{p}_bo"] / G).astype(f32)
        for i in (1, 2, 3):
            m[f"ln{i}_g"] = inp[f"ln{i}_g"].astype(f32)
            m[f"ln{i}_b"] = inp[f"ln{i}_b"].astype(f32)
        in_maps.append(m)
    return in_maps


_NC_CACHE = {}


def _get_nc(S):
    if S not in _NC_CACHE:
        _NC_CACHE[S] = build_decoder_nc(S)
    return _NC_CACHE[S]


def kernel(**inputs):
    x = np.asarray(inputs["input"])
    B, S, _ = x.shape
    nc = _get_nc(S)
    in_maps = shard_inputs(inputs)
    res = bass_utils.run_bass_kernel_spmd(nc, in_maps, core_ids=list(range(8)))
    outb = [np.concatenate([res.results[g * 4 + r]["out"] for r in range(4)], axis=0)
            for g in range(B)]
    return np.stack(outb, axis=0).astype(np.float32)

